# revision 16
# baseline (speedup 1.0000x reference)
"""AttentionLSTM Trainium2 kernel: data-parallel over batch on 8 NeuronCores.

Reference semantics (per batch element n):
  A_flat = A.reshape(N, H, 16); h0 = c0 = mean_p(A_flat)
  xWx = x @ Wx
  per step t:
    scores[p] = (h . A_flat[:, p]) / sqrt(H)
    w = softmax(scores); attn = A_flat @ w
    a = xWx_t + h @ Wh + attn @ Wattn + b
    i,f,o,g = sig/sig/sig/tanh of quarters; c = f*c + i*g; h = o*tanh(c)
  out[:, t, :] = h

Shapes: N=512, T=64, D=512, H=512 (4H=2048). 8 cores, 64 batch each.

Kernel mapping per core (n=64 local batch). All M=64 matmuls are run as
2x column-tiled pairs (tile_position (0,0)/(0,64)) so the full 128-wide
PE array is used; consequently every logical [64, 512] activation tensor
is stored FOLDED as [128, 256] (cols 0:256 -> partitions 0:64, cols
256:512 -> partitions 64:128), keeping ACT/DVE ops lane-aligned.

  - scores: PE cross-term X[(m-half), (n,p)] col-tiled pair into one
    [128, 512] PSUM bank, exp on ACT, diagonal via mask + strided reduce.
  - attn: block-diagonal matmul, stationary A_PT (const), moving wBD.
  - GEMM: a = [xT_t | hT | attnT]^T @ [Wx; Wh; Wattn] + ones1^T@b, each
    512-chunk computed as a (left-half, right-half) col-tiled pair of
    N=256 matmuls into a folded [128, 256] PSUM tile, 13 K-rounds.
  - gates on ACT from PSUM, state update on DVE, h transposes on PE.
"""

import math
import os
import sys

sys.path.insert(0, "/opt/trn_rl_repo")

import numpy as np
import ml_dtypes

import concourse.bass as bass
import concourse.mybir as mybir
from concourse.tile import TileContext
from concourse.bass_utils import run_bass_kernel_spmd

N, T, D, H = 512, 64, 512, 512
E = 4 * H  # 2048
NCORES = 8
NL = N // NCORES  # 64 batch per core
HF = H // 2  # 256, folded free size
P16 = 16  # attention positions
NB = 8  # batch blocks of 8 for block-diag attn
SCALE = 1.0 / math.sqrt(H)

F32 = mybir.dt.float32
# Matmul compute dtype. bf16 streams 1 row/cycle on the moving operand and
# is the well-trodden path on silicon; PSUM accumulation stays fp32.
MM_DT = mybir.dt.bfloat16


def build_nc(reps=1):
    nc = bass.Bass("TRN2", target_bir_lowering=False)

    # --- DRAM I/O ---
    xT_d = nc.declare_dram_parameter("xT", [T, D, NL], MM_DT, isOutput=False)
    AhT_d = nc.declare_dram_parameter("AhT", [H, NL * P16], MM_DT, isOutput=False)
    APT_d = nc.declare_dram_parameter("APT", [128, NB * H], MM_DT, isOutput=False)
    W_d = nc.declare_dram_parameter("W", [3 * H, E], MM_DT, isOutput=False)
    b_d = nc.declare_dram_parameter("bias", [1, E], MM_DT, isOutput=False)
    h0f_d = nc.declare_dram_parameter("h0f", [128, HF], F32, isOutput=False)
    h0T_d = nc.declare_dram_parameter("h0T", [H, NL], MM_DT, isOutput=False)
    i128_d = nc.declare_dram_parameter("i128", [128, 128], MM_DT, isOutput=False)
    i2x64_d = nc.declare_dram_parameter("i2x64", [128, NL], MM_DT, isOutput=False)
    d16_d = nc.declare_dram_parameter("d16", [P16, 128], MM_DT, isOutput=False)
    mask2f_d = nc.declare_dram_parameter("mask2f", [128, NL * 8], F32, isOutput=False)
    mBD64_d = nc.declare_dram_parameter("mBD64", [128, NL], MM_DT, isOutput=False)
    ones1_d = nc.declare_dram_parameter("ones1", [1, NL], MM_DT, isOutput=False)
    out_d = nc.declare_dram_parameter("out", [NL, T, H], F32, isOutput=True)

    with TileContext(nc) as tc:
        with (
            tc.tile_pool(name="wpool", bufs=1) as wpool,
            tc.tile_pool(name="state", bufs=1) as state,
            tc.tile_pool(name="xin", bufs=3) as xin,
            tc.tile_pool(name="work", bufs=2) as work,
            tc.tile_pool(name="hout", bufs=2) as hout,
            tc.tile_pool(name="psA", bufs=2, space="PSUM") as psA,
            tc.tile_pool(name="psG", bufs=1, space="PSUM") as psG,
            tc.tile_pool(name="psB", bufs=1, space="PSUM") as psB,
        ):
            # ---- persistent SBUF tensors ----
            W_sb = wpool.tile([128, 12, E], MM_DT, tag="W")  # 12 K-tiles of W
            nc.sync.dma_start(
                out=W_sb[:], in_=W_d.ap().rearrange("(k p) e -> p k e", p=128)
            )
            b_sb = wpool.tile([1, E], MM_DT, tag="bias")
            nc.sync.dma_start(out=b_sb[:], in_=b_d[:])
            AhT_sb = wpool.tile([128, 4, NL * P16], MM_DT, tag="AhT")
            nc.sync.dma_start(
                out=AhT_sb[:], in_=AhT_d.ap().rearrange("(k p) f -> p k f", p=128)
            )
            APT_sb = wpool.tile([128, NB, H], MM_DT, tag="APT")
            nc.sync.dma_start(
                out=APT_sb[:], in_=APT_d.ap().rearrange("p (b h) -> p b h", b=NB)
            )
            i128_sb = wpool.tile([128, 128], MM_DT, tag="i128")
            nc.sync.dma_start(out=i128_sb[:], in_=i128_d[:])
            i2x64_sb = wpool.tile([128, NL], MM_DT, tag="i2x64")
            nc.sync.dma_start(out=i2x64_sb[:], in_=i2x64_d[:])
            d16_sb = wpool.tile([P16, 128], MM_DT, tag="d16")
            nc.sync.dma_start(out=d16_sb[:], in_=d16_d[:])
            mask2f_sb = wpool.tile([128, NL * 8], F32, tag="mask2f")
            nc.sync.dma_start(out=mask2f_sb[:], in_=mask2f_d[:])
            mBD64_sb = wpool.tile([128, NL], MM_DT, tag="mBD64")
            nc.sync.dma_start(out=mBD64_sb[:], in_=mBD64_d[:])
            ones1_sb = wpool.tile([1, NL], MM_DT, tag="ones1")
            nc.sync.dma_start(out=ones1_sb[:], in_=ones1_d[:])

            # state: folded c (128, 256) and hT as 4 tiles (128, 64)
            cf_sb = state.tile([128, HF], F32, tag="cf")
            nc.sync.dma_start(out=cf_sb[:], in_=h0f_d[:])
            hT_sb = state.tile([128, 4, NL], MM_DT, tag="hT")
            nc.sync.dma_start(
                out=hT_sb[:], in_=h0T_d.ap().rearrange("(k p) n -> p k n", p=128)
            )

            _lp = tc.For_i(0, reps, 1) if reps > 1 else None
            if _lp is not None:
                _lp.__enter__()
            for t in range(T):
                # ---- stream x_t^T (512, 64) as 4 K-tiles ----
                xTt = xin.tile([128, 4, NL], MM_DT, tag="xT")
                nc.sync.dma_start(
                    out=xTt[:],
                    in_=xT_d[t].rearrange("(k p) n -> p k n", p=128),
                )

                # ---- scores + softmax + attn build: this whole chain is the
                # per-step critical path -> high scheduler priority so the
                # X matmuls issue first and the chain overlaps the hx GEMM.
                with tc.high_priority():
                    # 1) scores cross-term, col-tiled pair: rows 0:64 =
                    #    X[m, (n<32, p)], rows 64:128 = X[m, (n>=32, p)]
                    X_ps = psA.tile([128, NL * 8], F32, tag="X")
                    for k in range(4):
                        nc.tensor.matmul(
                            X_ps[0:NL, :],
                            hT_sb[:, k],
                            AhT_sb[:, k, 0:512],
                            start=(k == 0),
                            stop=(k == 3),
                            tile_position=(0, 0),
                        )
                        nc.tensor.matmul(
                            X_ps[NL:128, :],
                            hT_sb[:, k],
                            AhT_sb[:, k, 512:1024],
                            start=(k == 0),
                            stop=(k == 3),
                            tile_position=(0, 64),
                        )
                    # 2) diagonal extract on raw scores: mask then strided
                    #    reduce over n -> scores rows {0:32, 96:128}
                    Xm = work.tile([128, NL * 8], F32, tag="Xm")
                    nc.vector.tensor_mul(Xm[:], X_ps[:], mask2f_sb[:])
                    scS = work.tile([128, P16], F32, tag="scS")
                    nc.vector.reduce_sum(
                        scS[:],
                        Xm[:].rearrange("q (n p) -> q p n", p=P16),
                        axis=mybir.AxisListType.X,
                    )
                    # 3) exp via sigmoid (stays on the sigmoid ACT table:
                    #    e^x = sig(x)/(1-sig(x)); softmax is shift-free)
                    sg = work.tile([128, P16], F32, tag="sg")
                    nc.scalar.activation(
                        sg[:], scS[:], mybir.ActivationFunctionType.Sigmoid,
                        scale=SCALE,
                    )
                    om = work.tile([128, P16], F32, tag="om")
                    nc.vector.tensor_scalar(
                        om[:], sg[:], -1.0, 1.0,
                        op0=mybir.AluOpType.mult, op1=mybir.AluOpType.add,
                    )
                    omr = work.tile([128, P16], F32, tag="omr")
                    nc.vector.reciprocal(omr[:], om[:])
                    expS = work.tile([128, P16], F32, tag="expS")
                    nc.vector.tensor_mul(expS[:], sg[:], omr[:])
                    # 5) denom, reciprocal
                    den = work.tile([128, 1], F32, tag="den")
                    nc.vector.reduce_sum(den[:], expS[:], axis=mybir.AxisListType.X)
                    rd = work.tile([128, 1], F32, tag="rd")
                    nc.vector.reciprocal(rd[:], den[:])
                    # 6) normalize: wS = expS * (1/den), cast to bf16
                    wS = work.tile([128, P16], MM_DT, tag="wS")
                    nc.vector.tensor_scalar_mul(wS[:], expS[:], rd[:])
                    # 7) transpose wS -> (16, 128); cols {0:32, 96:128} valid;
                    #    replicate to (128, 64) picking the valid cols
                    small_ps = psB.tile([128, 512], F32, tag="small")
                    eT_ps = small_ps[:P16, 0:128].bitcast(MM_DT)[:, 0:128]
                    nc.tensor.transpose(eT_ps, wS[:], i128_sb[:])
                    wST = work.tile([P16, 128], MM_DT, tag="wST")
                    nc.scalar.copy(wST[:], eT_ps)
                    rep_ps = small_ps[:, 128:192]
                    nc.tensor.matmul(
                        rep_ps[:, 0:32], d16_sb[:], wST[:, 0:32],
                        start=True, stop=True,
                    )
                    nc.tensor.matmul(
                        rep_ps[:, 32:64], d16_sb[:], wST[:, 96:128],
                        start=True, stop=True,
                    )
                    # 8) wBD = rep * mask64 straight from PSUM
                    wBD = work.tile([128, NL], MM_DT, tag="wBD")
                    nc.vector.tensor_mul(wBD[:], rep_ps[:], mBD64_sb[:])

                    # 9) attnT directly: stationary = A_PT slices (const),
                    #    moving = wBD 8-col blocks -> attnT_ps[j] (128, 64).
                    #    Copy out per j so attn-part matmuls pipeline.
                    attnT = work.tile([128, 4, NL], MM_DT, tag="attnT")
                    at_ps = psB.tile([128, 4, NL], F32, tag="atps")
                    for j in range(4):
                        for bb in range(NB):
                            nc.tensor.matmul(
                                at_ps[:, j, bb * 8 : (bb + 1) * 8],
                                APT_sb[:, bb, j * 128 : (j + 1) * 128],
                                wBD[:, bb * 8 : (bb + 1) * 8],
                                start=True,
                                stop=True,
                            )
                        nc.scalar.copy(attnT[:, j], at_ps[:, j])

                # ---- 15) big GEMM: a = [xT|hT|attnT|ones]^T @ [W; b] ----
                # Each 512-chunk is a col-tiled pair of N=256 matmuls into a
                # folded [128, 256] PSUM tile. hx-parts first (no attention
                # dependency): chunk order f,i,g,o so gates/c-update pipeline.
                CHUNK_ORDER = (1, 0, 3, 2)  # f, i, g, o quarters of [i|f|o|g]
                # one PSUM bank per chunk (start=True clears the whole bank,
                # so chunks must not share banks)
                a_ps = {}
                for nck in CHUNK_ORDER:
                    a_ps[nck] = psG.tile([128, HF], F32, tag=f"a{nck}", name=f"a{nck}")

                def cslr(nck):
                    return (
                        slice(nck * 512, nck * 512 + HF),
                        slice(nck * 512 + HF, (nck + 1) * 512),
                    )

                def gemm_round(stat, wrow, nck, start, stop):
                    csl, csr = cslr(nck)
                    nc.tensor.matmul(
                        a_ps[nck][0:NL, :], stat, W_sb[:, wrow, csl],
                        start=start, stop=stop, tile_position=(0, 0),
                    )
                    nc.tensor.matmul(
                        a_ps[nck][NL:128, :], stat, W_sb[:, wrow, csr],
                        start=start, stop=stop, tile_position=(0, 64),
                    )

                # hx part, k-major so each stationary is loaded once per
                # 4-chunk sweep and LDWEIGHTS fully hides under streaming
                for k in range(4):
                    for nck in CHUNK_ORDER:
                        gemm_round(xTt[:, k], k, nck, start=(k == 0), stop=False)
                for k in range(4):
                    for nck in CHUNK_ORDER:
                        gemm_round(hT_sb[:, k], 4 + k, nck, start=False, stop=False)
                for nck in CHUNK_ORDER:
                    csl, csr = cslr(nck)
                    nc.tensor.matmul(
                        a_ps[nck][0:NL, :], ones1_sb[:], b_sb[:, csl],
                        start=False, stop=False, tile_position=(0, 0),
                    )
                    nc.tensor.matmul(
                        a_ps[nck][NL:128, :], ones1_sb[:], b_sb[:, csr],
                        start=False, stop=False, tile_position=(0, 64),
                    )
                ig = work.tile([128, HF], F32, tag="ig")
                fg = work.tile([128, HF], F32, tag="fg")
                og = work.tile([128, HF], F32, tag="og")
                gg = work.tile([128, HF], F32, tag="gg")
                igp = work.tile([128, HF], F32, tag="igp")
                fcp = work.tile([128, HF], F32, tag="fcp")
                tc_sb = work.tile([128, HF], F32, tag="tc")
                hN = hout.tile([128, HF], F32, tag="hN")
                Sig = mybir.ActivationFunctionType.Sigmoid
                Tanh = mybir.ActivationFunctionType.Tanh
                # attn part: (f,i) k-major, gates f/i, then (g,o) k-major so
                # the f/i gate work overlaps the g/o matmuls
                for k in range(4):
                    for nck in (1, 0):
                        gemm_round(attnT[:, k], 8 + k, nck,
                                   start=False, stop=(k == 3))
                nc.scalar.activation(fg[:], a_ps[1][:], Sig)
                nc.scalar.activation(ig[:], a_ps[0][:], Sig)
                nc.vector.tensor_mul(fcp[:], fg[:], cf_sb[:])
                for k in range(4):
                    for nck in (3, 2):
                        gemm_round(attnT[:, k], 8 + k, nck,
                                   start=False, stop=(k == 3))
                nc.scalar.activation(gg[:], a_ps[3][:], Tanh)
                nc.vector.tensor_mul(igp[:], ig[:], gg[:])
                nc.scalar.activation(og[:], a_ps[2][:], Sig)
                nc.vector.tensor_add(cf_sb[:], fcp[:], igp[:])
                nc.scalar.activation(tc_sb[:], cf_sb[:], Tanh)
                # tail: h = o * tanh(c) in 2 folded halves; bf16 cast on ACT,
                # transposes on PE, evac on DVE so next-step scores start asap
                hNb = hout.tile([128, HF], MM_DT, tag="hNb")
                for u in range(2):
                    us = slice(u * 128, (u + 1) * 128)
                    nc.vector.tensor_mul(hN[:, us], og[:, us], tc_sb[:, us])
                    if t < T - 1:
                        nc.scalar.copy(hNb[:, us], hN[:, us])
                        for v in range(2):  # j = u + 2v
                            j = u + 2 * v
                            rs = slice(v * NL, (v + 1) * NL)
                            tp_ps = (
                                small_ps[:, 192 + 64 * j : 256 + 64 * j]
                                .bitcast(MM_DT)[:, 0:NL]
                            )
                            nc.tensor.transpose(
                                tp_ps, hNb[rs, us], i2x64_sb[rs, :]
                            )
                            nc.vector.tensor_copy(hT_sb[:, j], tp_ps)

                # ---- 19) DMA out (unfold halves) ----
                nc.sync.dma_start(out=out_d[:, t, 0:HF], in_=hN[0:NL, :])
                nc.sync.dma_start(out=out_d[:, t, HF:H], in_=hN[NL:128, :])

            if _lp is not None:
                _lp.__exit__(None, None, None)

    _split_matmul_waits(nc)
    return nc


def _split_matmul_waits(nc):
    """Several TPB instruction encodings accept only one sync-wait command;
    hoist excess waits onto an inserted same-engine drain."""
    cnt = 0
    for f in nc.m.functions:
        for blk in f.blocks:
            new_insts = []
            for ins in blk.instructions:
                if (
                    ins.sync_info is not None
                    and ins.sync_info.on_wait
                    and len(ins.sync_info.on_wait) > 1
                ):
                    waits = list(ins.sync_info.on_wait)
                    for w in waits[:-1]:
                        cnt += 1
                        d = mybir.InstDrain(
                            name=f"I-mmw{cnt}", ins=[], outs=[],
                            engine=ins.engine,
                        )
                        d.sync_info = mybir.SyncInfo(on_wait=[w], on_update=[])
                        new_insts.append(d)
                    ins.sync_info = mybir.SyncInfo(
                        on_wait=[waits[-1]], on_update=list(ins.sync_info.on_update or [])
                    )
                new_insts.append(ins)
            blk.instructions = new_insts


def _prep_core_inputs(x_i, A_i, Wx, Wh, Wattn, b):
    """Host-side layout prep for one core's shard (x_i: (64,T,D), A_i: (64,H,4,4))."""
    nl = x_i.shape[0]
    A_flat = A_i.reshape(nl, H, P16)
    h0 = A_flat.mean(axis=2).astype(np.float32)  # (64, H)

    xT = np.ascontiguousarray(x_i.transpose(1, 2, 0)).astype(np.float32)  # (T, D, 64)
    AhT = np.ascontiguousarray(
        A_flat.transpose(1, 0, 2).reshape(H, nl * P16)
    ).astype(np.float32)
    # APT[(p, n_sub), (b, h)] = A_flat[8b + n_sub, h, p]
    APT = np.ascontiguousarray(
        A_flat.reshape(NB, 8, H, P16).transpose(3, 1, 0, 2).reshape(128, NB * H)
    ).astype(np.float32)
    W = np.concatenate([Wx, Wh, Wattn], axis=0).astype(np.float32)  # (1536, E)
    i128 = np.eye(128, dtype=np.float32)
    i2x64 = np.concatenate([np.eye(NL), np.eye(NL)], axis=0).astype(np.float32)
    d16 = np.repeat(np.eye(P16, dtype=np.float32), 8, axis=1)  # (16, 128)
    # mask2f[q, 16*n' + p]: left half (n<32) diag lives at rows 0:32,
    # right half (n>=32) at rows 96:128.
    mask2f = np.zeros((128, NL * 8), dtype=np.float32)
    for n in range(32):
        mask2f[n, 16 * n : 16 * n + 16] = 1.0
    for n in range(32, NL):
        mask2f[64 + n, 16 * (n - 32) : 16 * (n - 32) + 16] = 1.0
    mBD64 = np.tile(np.tile(np.eye(8, dtype=np.float32), (1, 8)), (P16, 1))  # (128,64)
    ones1 = np.ones((1, NL), dtype=np.float32)
    h0f = np.concatenate([h0[:, :HF], h0[:, HF:]], axis=0)  # (128, 256) folded
    bf16 = ml_dtypes.bfloat16
    return {
        "xT": xT.astype(bf16),
        "AhT": AhT.astype(bf16),
        "APT": APT.astype(bf16),
        "W": W.astype(bf16),
        "bias": b.reshape(1, E).astype(bf16),
        "h0f": h0f,
        "h0T": np.ascontiguousarray(h0.T).astype(bf16),
        "i128": i128.astype(bf16),
        "i2x64": i2x64.astype(bf16),
        "d16": d16.astype(bf16),
        "mask2f": mask2f,
        "mBD64": mBD64.astype(bf16),
        "ones1": ones1.astype(bf16),
    }


_NC_CACHE = {}


def kernel(x, A, Wx, Wh, Wattn, b, _trace=False):
    x = np.asarray(x, dtype=np.float32)
    A = np.asarray(A, dtype=np.float32)
    Wx = np.asarray(Wx, dtype=np.float32)
    Wh = np.asarray(Wh, dtype=np.float32)
    Wattn = np.asarray(Wattn, dtype=np.float32)
    b = np.asarray(b, dtype=np.float32)

    if "nc" not in _NC_CACHE:
        _NC_CACHE["nc"] = build_nc()
    nc = _NC_CACHE["nc"]

    in_maps = []
    for i in range(NCORES):
        sl = slice(i * NL, (i + 1) * NL)
        in_maps.append(_prep_core_inputs(x[sl], A[sl], Wx, Wh, Wattn, b))

    res = run_bass_kernel_spmd(
        nc, in_maps, core_ids=list(range(NCORES)), trace=_trace
    )
    outs = [res.results[i]["out"] for i in range(NCORES)]
    full = np.concatenate(outs, axis=0)  # (N, T, H)
    if _trace:
        kernel.last_exec_time_ns = res.exec_time_ns
        kernel.last_profile = res.profile_json
    return full


kernel.last_exec_time_ns = None
kernel.last_profile = None


# revision 19
# speedup vs baseline: 1.0872x; 1.0872x over previous
"""AttentionLSTM Trainium2 kernel: data-parallel over batch on 8 NeuronCores.

Reference semantics (per batch element n):
  A_flat = A.reshape(N, H, 16); h0 = c0 = mean_p(A_flat)
  xWx = x @ Wx
  per step t:
    scores[p] = (h . A_flat[:, p]) / sqrt(H)
    w = softmax(scores); attn = A_flat @ w
    a = xWx_t + h @ Wh + attn @ Wattn + b
    i,f,o,g = sig/sig/sig/tanh of quarters; c = f*c + i*g; h = o*tanh(c)
  out[:, t, :] = h

Shapes: N=512, T=64, D=512, H=512 (4H=2048). 8 cores, 64 batch each.

Kernel mapping per core (n=64 local batch). All M=64 matmuls are run as
2x column-tiled pairs (tile_position (0,0)/(0,64)) so the full 128-wide
PE array is used; consequently every logical [64, 512] activation tensor
is stored FOLDED as [128, 256] (cols 0:256 -> partitions 0:64, cols
256:512 -> partitions 64:128), keeping ACT/DVE ops lane-aligned.

  - scores: PE cross-term X[(m-half), (n,p)] col-tiled pair into one
    [128, 512] PSUM bank, exp on ACT, diagonal via mask + strided reduce.
  - attn: block-diagonal matmul, stationary A_PT (const), moving wBD.
  - GEMM: a = [xT_t | hT | attnT]^T @ [Wx; Wh; Wattn] + ones1^T@b, each
    512-chunk computed as a (left-half, right-half) col-tiled pair of
    N=256 matmuls into a folded [128, 256] PSUM tile, 13 K-rounds.
  - gates on ACT from PSUM, state update on DVE, h transposes on PE.
"""

import math
import os
import sys

sys.path.insert(0, "/opt/trn_rl_repo")

import numpy as np
import ml_dtypes

import concourse.bass as bass
import concourse.mybir as mybir
from concourse.tile import TileContext
from concourse.bass_utils import run_bass_kernel_spmd

N, T, D, H = 512, 64, 512, 512
E = 4 * H  # 2048
NCORES = 8
NL = N // NCORES  # 64 batch per core
HF = H // 2  # 256, folded free size
P16 = 16  # attention positions
NB = 8  # batch blocks of 8 for block-diag attn
SCALE = 1.0 / math.sqrt(H)

F32 = mybir.dt.float32
# Matmul compute dtype. bf16 streams 1 row/cycle on the moving operand and
# is the well-trodden path on silicon; PSUM accumulation stays fp32.
MM_DT = mybir.dt.bfloat16


def build_nc(reps=1):
    nc = bass.Bass("TRN2", target_bir_lowering=False)

    # --- DRAM I/O ---
    xT_d = nc.declare_dram_parameter("xT", [T, D, NL], MM_DT, isOutput=False)
    AhT_d = nc.declare_dram_parameter("AhT", [H, NL * P16], MM_DT, isOutput=False)
    APT_d = nc.declare_dram_parameter("APT", [128, NB * H], MM_DT, isOutput=False)
    W_d = nc.declare_dram_parameter("W", [3 * H, E], MM_DT, isOutput=False)
    b_d = nc.declare_dram_parameter("bias", [1, E], MM_DT, isOutput=False)
    h0f_d = nc.declare_dram_parameter("h0f", [128, HF], F32, isOutput=False)
    h0T_d = nc.declare_dram_parameter("h0T", [H, NL], MM_DT, isOutput=False)
    i128_d = nc.declare_dram_parameter("i128", [128, 128], MM_DT, isOutput=False)
    i2x64_d = nc.declare_dram_parameter("i2x64", [128, NL], MM_DT, isOutput=False)
    d16_d = nc.declare_dram_parameter("d16", [P16, 128], MM_DT, isOutput=False)
    mask2f_d = nc.declare_dram_parameter("mask2f", [128, NL * 8], F32, isOutput=False)
    mBD64_d = nc.declare_dram_parameter("mBD64", [128, NL], MM_DT, isOutput=False)
    ones1_d = nc.declare_dram_parameter("ones1", [1, NL], MM_DT, isOutput=False)
    out_d = nc.declare_dram_parameter("out", [NL, T, H], F32, isOutput=True)

    with TileContext(nc) as tc:
        with (
            tc.tile_pool(name="wpool", bufs=1) as wpool,
            tc.tile_pool(name="state", bufs=1) as state,
            tc.tile_pool(name="xin", bufs=3) as xin,
            tc.tile_pool(name="work", bufs=2) as work,
            tc.tile_pool(name="hout", bufs=2) as hout,
            tc.tile_pool(name="psA", bufs=1, space="PSUM") as psA,
            tc.tile_pool(name="psG", bufs=1, space="PSUM") as psG,
            tc.tile_pool(name="psB", bufs=1, space="PSUM") as psB,
        ):
            # ---- persistent SBUF tensors ----
            W_sb = wpool.tile([128, 12, E], MM_DT, tag="W")  # 12 K-tiles of W
            nc.sync.dma_start(
                out=W_sb[:], in_=W_d.ap().rearrange("(k p) e -> p k e", p=128)
            )
            b_sb = wpool.tile([1, E], MM_DT, tag="bias")
            nc.sync.dma_start(out=b_sb[:], in_=b_d[:])
            AhT_sb = wpool.tile([128, 4, NL * P16], MM_DT, tag="AhT")
            nc.sync.dma_start(
                out=AhT_sb[:], in_=AhT_d.ap().rearrange("(k p) f -> p k f", p=128)
            )
            APT_sb = wpool.tile([128, NB, H], MM_DT, tag="APT")
            nc.sync.dma_start(
                out=APT_sb[:], in_=APT_d.ap().rearrange("p (b h) -> p b h", b=NB)
            )
            i128_sb = wpool.tile([128, 128], MM_DT, tag="i128")
            nc.sync.dma_start(out=i128_sb[:], in_=i128_d[:])
            i2x64_sb = wpool.tile([128, NL], MM_DT, tag="i2x64")
            nc.sync.dma_start(out=i2x64_sb[:], in_=i2x64_d[:])
            d16_sb = wpool.tile([P16, 128], MM_DT, tag="d16")
            nc.sync.dma_start(out=d16_sb[:], in_=d16_d[:])
            mask2f_sb = wpool.tile([128, NL * 8], F32, tag="mask2f")
            nc.sync.dma_start(out=mask2f_sb[:], in_=mask2f_d[:])
            mBD64_sb = wpool.tile([128, NL], MM_DT, tag="mBD64")
            nc.sync.dma_start(out=mBD64_sb[:], in_=mBD64_d[:])
            ones1_sb = wpool.tile([1, NL], MM_DT, tag="ones1")
            nc.sync.dma_start(out=ones1_sb[:], in_=ones1_d[:])

            # state: folded c (128, 256) and hT as 4 tiles (128, 64)
            cf_sb = state.tile([128, HF], F32, tag="cf")
            nc.sync.dma_start(out=cf_sb[:], in_=h0f_d[:])
            hT_sb = state.tile([128, 4, NL], MM_DT, tag="hT")
            nc.sync.dma_start(
                out=hT_sb[:], in_=h0T_d.ap().rearrange("(k p) n -> p k n", p=128)
            )

            _lp = tc.For_i(0, reps, 1) if reps > 1 else None
            if _lp is not None:
                _lp.__enter__()
            for t in range(T):
                # ---- stream x_t^T (512, 64) as 4 K-tiles ----
                xTt = xin.tile([128, 4, NL], MM_DT, tag="xT")
                nc.sync.dma_start(
                    out=xTt[:],
                    in_=xT_d[t].rearrange("(k p) n -> p k n", p=128),
                )

                # ---- scores + softmax + attn build: this whole chain is the
                # per-step critical path -> high scheduler priority so the
                # X matmuls issue first and the chain overlaps the hx GEMM.
                with tc.high_priority():
                    # 1) scores cross-term, col-tiled pair: rows 0:64 =
                    #    X[m, (n<32, p)], rows 64:128 = X[m, (n>=32, p)]
                    X_ps = psA.tile([128, NL * 8], F32, tag="X")
                    for k in range(4):
                        nc.tensor.matmul(
                            X_ps[0:NL, :],
                            hT_sb[:, k],
                            AhT_sb[:, k, 0:512],
                            start=(k == 0),
                            stop=(k == 3),
                            tile_position=(0, 0),
                        )
                        nc.tensor.matmul(
                            X_ps[NL:128, :],
                            hT_sb[:, k],
                            AhT_sb[:, k, 512:1024],
                            start=(k == 0),
                            stop=(k == 3),
                            tile_position=(0, 64),
                        )
                    # 2) diagonal extract on raw scores: mask then strided
                    #    reduce over n -> scores rows {0:32, 96:128}
                    Xm = work.tile([128, NL * 8], F32, tag="Xm")
                    nc.vector.tensor_mul(Xm[:], X_ps[:], mask2f_sb[:])
                    scS = work.tile([128, P16], F32, tag="scS")
                    nc.vector.reduce_sum(
                        scS[:],
                        Xm[:].rearrange("q (n p) -> q p n", p=P16),
                        axis=mybir.AxisListType.X,
                    )
                    # 3) exp via sigmoid (stays on the sigmoid ACT table:
                    #    e^x = sig(x)/(1-sig(x)); softmax is shift-free)
                    sg = work.tile([128, P16], F32, tag="sg")
                    nc.scalar.activation(
                        sg[:], scS[:], mybir.ActivationFunctionType.Sigmoid,
                        scale=SCALE,
                    )
                    om = work.tile([128, P16], F32, tag="om")
                    nc.vector.tensor_scalar(
                        om[:], sg[:], -1.0, 1.0,
                        op0=mybir.AluOpType.mult, op1=mybir.AluOpType.add,
                    )
                    omr = work.tile([128, P16], F32, tag="omr")
                    nc.vector.reciprocal(omr[:], om[:])
                    expS = work.tile([128, P16], F32, tag="expS")
                    nc.vector.tensor_mul(expS[:], sg[:], omr[:])
                    # 5) denom, reciprocal
                    den = work.tile([128, 1], F32, tag="den")
                    nc.vector.reduce_sum(den[:], expS[:], axis=mybir.AxisListType.X)
                    rd = work.tile([128, 1], F32, tag="rd")
                    nc.vector.reciprocal(rd[:], den[:])
                    # 6) normalize: wS = expS * (1/den), cast to bf16
                    wS = work.tile([128, P16], MM_DT, tag="wS")
                    nc.vector.tensor_scalar_mul(wS[:], expS[:], rd[:])
                    # 7) transpose wS -> (16, 128); cols {0:32, 96:128} valid;
                    #    replicate to (128, 64) picking the valid cols
                    small_ps = psB.tile([128, 512], F32, tag="small")
                    eT_ps = small_ps[:P16, 0:128].bitcast(MM_DT)[:, 0:128]
                    nc.tensor.transpose(eT_ps, wS[:], i128_sb[:])
                    wST = work.tile([P16, 128], MM_DT, tag="wST")
                    nc.scalar.copy(wST[:], eT_ps)
                    rep_ps = small_ps[:, 128:192]
                    nc.tensor.matmul(
                        rep_ps[:, 0:32], d16_sb[:], wST[:, 0:32],
                        start=True, stop=True,
                    )
                    nc.tensor.matmul(
                        rep_ps[:, 32:64], d16_sb[:], wST[:, 96:128],
                        start=True, stop=True,
                    )
                    # 8) wBD = rep * mask64 straight from PSUM
                    wBD = work.tile([128, NL], MM_DT, tag="wBD")
                    nc.vector.tensor_mul(wBD[:], rep_ps[:], mBD64_sb[:])

                    # 9) attnT directly: stationary = A_PT slices (const),
                    #    moving = wBD 8-col blocks -> attnT_ps[j] (128, 64).
                    #    j 0-1 and j 2-3 land in different PSUM banks so the
                    #    first copy (DVE) runs while the j 2-3 matmuls stream;
                    #    second bank copied on ACT in parallel.
                    attnT = work.tile([128, 4, NL], MM_DT, tag="attnT")
                    at_psA = psB.tile([128, 2, NL], F32, tag="atpsA")
                    at_psB = psB.tile([128, 2, NL], F32, tag="atpsB")
                    for j in range(4):
                        dst = at_psA if j < 2 else at_psB
                        for bb in range(NB):
                            nc.tensor.matmul(
                                dst[:, j % 2, bb * 8 : (bb + 1) * 8],
                                APT_sb[:, bb, j * 128 : (j + 1) * 128],
                                wBD[:, bb * 8 : (bb + 1) * 8],
                                start=True,
                                stop=True,
                            )
                        if j == 1:
                            nc.vector.tensor_copy(attnT[:, 0:2], at_psA[:])
                        elif j == 3:
                            nc.scalar.copy(attnT[:, 2:4], at_psB[:])

                # ---- 15) big GEMM: a = [xT|hT|attnT|ones]^T @ [W; b] ----
                # Each 512-chunk is a col-tiled pair of N=256 matmuls into a
                # folded [128, 256] PSUM tile. hx-parts first (no attention
                # dependency): chunk order f,i,g,o so gates/c-update pipeline.
                CHUNK_ORDER = (1, 0, 3, 2)  # f, i, g, o quarters of [i|f|o|g]
                # one PSUM bank per chunk (start=True clears the whole bank,
                # so chunks must not share banks)
                a_ps = {}
                for nck in CHUNK_ORDER:
                    a_ps[nck] = psG.tile([128, HF], F32, tag=f"a{nck}", name=f"a{nck}")

                def cslr(nck):
                    return (
                        slice(nck * 512, nck * 512 + HF),
                        slice(nck * 512 + HF, (nck + 1) * 512),
                    )

                def gemm_round(stat, wrow, nck, start, stop):
                    csl, csr = cslr(nck)
                    nc.tensor.matmul(
                        a_ps[nck][0:NL, :], stat, W_sb[:, wrow, csl],
                        start=start, stop=stop, tile_position=(0, 0),
                    )
                    nc.tensor.matmul(
                        a_ps[nck][NL:128, :], stat, W_sb[:, wrow, csr],
                        start=start, stop=stop, tile_position=(0, 64),
                    )

                # hx part, k-major so each stationary is loaded once per
                # 4-chunk sweep and LDWEIGHTS fully hides under streaming
                for k in range(4):
                    for nck in CHUNK_ORDER:
                        gemm_round(xTt[:, k], k, nck, start=(k == 0), stop=False)
                for k in range(4):
                    for nck in CHUNK_ORDER:
                        gemm_round(hT_sb[:, k], 4 + k, nck, start=False, stop=False)
                for nck in CHUNK_ORDER:
                    csl, csr = cslr(nck)
                    nc.tensor.matmul(
                        a_ps[nck][0:NL, :], ones1_sb[:], b_sb[:, csl],
                        start=False, stop=False, tile_position=(0, 0),
                    )
                    nc.tensor.matmul(
                        a_ps[nck][NL:128, :], ones1_sb[:], b_sb[:, csr],
                        start=False, stop=False, tile_position=(0, 64),
                    )
                ig = work.tile([128, HF], F32, tag="ig")
                fg = work.tile([128, HF], F32, tag="fg")
                og = work.tile([128, HF], F32, tag="og")
                gg = work.tile([128, HF], F32, tag="gg")
                igp = work.tile([128, HF], F32, tag="igp")
                fcp = work.tile([128, HF], F32, tag="fcp")
                tc_sb = work.tile([128, HF], F32, tag="tc")
                hN = hout.tile([128, HF], F32, tag="hN")
                Sig = mybir.ActivationFunctionType.Sigmoid
                Tanh = mybir.ActivationFunctionType.Tanh
                # attn part: (f,i) k-major, gates f/i, then (g,o) k-major so
                # the f/i gate work overlaps the g/o matmuls
                for k in range(4):
                    for nck in (1, 0):
                        gemm_round(attnT[:, k], 8 + k, nck,
                                   start=False, stop=(k == 3))
                nc.scalar.activation(fg[:], a_ps[1][:], Sig)
                nc.scalar.activation(ig[:], a_ps[0][:], Sig)
                nc.vector.tensor_mul(fcp[:], fg[:], cf_sb[:])
                for k in range(4):
                    for nck in (3, 2):
                        gemm_round(attnT[:, k], 8 + k, nck,
                                   start=False, stop=(k == 3))
                nc.scalar.activation(gg[:], a_ps[3][:], Tanh)
                nc.scalar.activation(og[:], a_ps[2][:], Sig)
                # c-chain sliced in halves so tanh(c) (and then h/hT) for the
                # first half starts while the second half is still in flight
                for u in range(2):
                    us = slice(u * 128, (u + 1) * 128)
                    nc.vector.tensor_mul(igp[:, us], ig[:, us], gg[:, us])
                    nc.vector.tensor_add(cf_sb[:, us], fcp[:, us], igp[:, us])
                    nc.scalar.activation(tc_sb[:, us], cf_sb[:, us], Tanh)
                # tail: h = o * tanh(c) in 2 folded halves; bf16 cast on ACT,
                # transposes on PE, evac on DVE so next-step scores start asap
                hNb = hout.tile([128, HF], MM_DT, tag="hNb")
                for u in range(2):
                    us = slice(u * 128, (u + 1) * 128)
                    nc.vector.tensor_mul(hN[:, us], og[:, us], tc_sb[:, us])
                    if t < T - 1:
                        nc.scalar.copy(hNb[:, us], hN[:, us])
                        for v in range(2):  # j = u + 2v
                            j = u + 2 * v
                            rs = slice(v * NL, (v + 1) * NL)
                            tp_ps = (
                                small_ps[:, 192 + 64 * j : 256 + 64 * j]
                                .bitcast(MM_DT)[:, 0:NL]
                            )
                            nc.tensor.transpose(
                                tp_ps, hNb[rs, us], i2x64_sb[rs, :]
                            )
                            nc.vector.tensor_copy(hT_sb[:, j], tp_ps)

                # ---- 19) DMA out (unfold halves) ----
                nc.sync.dma_start(out=out_d[:, t, 0:HF], in_=hN[0:NL, :])
                nc.sync.dma_start(out=out_d[:, t, HF:H], in_=hN[NL:128, :])

            if _lp is not None:
                _lp.__exit__(None, None, None)

    _split_matmul_waits(nc)
    return nc


def _split_matmul_waits(nc):
    """Several TPB instruction encodings accept only one sync-wait command;
    hoist excess waits onto an inserted same-engine drain."""
    cnt = 0
    for f in nc.m.functions:
        for blk in f.blocks:
            new_insts = []
            for ins in blk.instructions:
                if (
                    ins.sync_info is not None
                    and ins.sync_info.on_wait
                    and len(ins.sync_info.on_wait) > 1
                ):
                    waits = list(ins.sync_info.on_wait)
                    for w in waits[:-1]:
                        cnt += 1
                        d = mybir.InstDrain(
                            name=f"I-mmw{cnt}", ins=[], outs=[],
                            engine=ins.engine,
                        )
                        d.sync_info = mybir.SyncInfo(on_wait=[w], on_update=[])
                        new_insts.append(d)
                    ins.sync_info = mybir.SyncInfo(
                        on_wait=[waits[-1]], on_update=list(ins.sync_info.on_update or [])
                    )
                new_insts.append(ins)
            blk.instructions = new_insts


def _prep_core_inputs(x_i, A_i, Wx, Wh, Wattn, b):
    """Host-side layout prep for one core's shard (x_i: (64,T,D), A_i: (64,H,4,4))."""
    nl = x_i.shape[0]
    A_flat = A_i.reshape(nl, H, P16)
    h0 = A_flat.mean(axis=2).astype(np.float32)  # (64, H)

    xT = np.ascontiguousarray(x_i.transpose(1, 2, 0)).astype(np.float32)  # (T, D, 64)
    AhT = np.ascontiguousarray(
        A_flat.transpose(1, 0, 2).reshape(H, nl * P16)
    ).astype(np.float32)
    # APT[(p, n_sub), (b, h)] = A_flat[8b + n_sub, h, p]
    APT = np.ascontiguousarray(
        A_flat.reshape(NB, 8, H, P16).transpose(3, 1, 0, 2).reshape(128, NB * H)
    ).astype(np.float32)
    W = np.concatenate([Wx, Wh, Wattn], axis=0).astype(np.float32)  # (1536, E)
    i128 = np.eye(128, dtype=np.float32)
    i2x64 = np.concatenate([np.eye(NL), np.eye(NL)], axis=0).astype(np.float32)
    d16 = np.repeat(np.eye(P16, dtype=np.float32), 8, axis=1)  # (16, 128)
    # mask2f[q, 16*n' + p]: left half (n<32) diag lives at rows 0:32,
    # right half (n>=32) at rows 96:128.
    mask2f = np.zeros((128, NL * 8), dtype=np.float32)
    for n in range(32):
        mask2f[n, 16 * n : 16 * n + 16] = 1.0
    for n in range(32, NL):
        mask2f[64 + n, 16 * (n - 32) : 16 * (n - 32) + 16] = 1.0
    mBD64 = np.tile(np.tile(np.eye(8, dtype=np.float32), (1, 8)), (P16, 1))  # (128,64)
    ones1 = np.ones((1, NL), dtype=np.float32)
    h0f = np.concatenate([h0[:, :HF], h0[:, HF:]], axis=0)  # (128, 256) folded
    bf16 = ml_dtypes.bfloat16
    return {
        "xT": xT.astype(bf16),
        "AhT": AhT.astype(bf16),
        "APT": APT.astype(bf16),
        "W": W.astype(bf16),
        "bias": b.reshape(1, E).astype(bf16),
        "h0f": h0f,
        "h0T": np.ascontiguousarray(h0.T).astype(bf16),
        "i128": i128.astype(bf16),
        "i2x64": i2x64.astype(bf16),
        "d16": d16.astype(bf16),
        "mask2f": mask2f,
        "mBD64": mBD64.astype(bf16),
        "ones1": ones1.astype(bf16),
    }


_NC_CACHE = {}


def kernel(x, A, Wx, Wh, Wattn, b, _trace=False):
    x = np.asarray(x, dtype=np.float32)
    A = np.asarray(A, dtype=np.float32)
    Wx = np.asarray(Wx, dtype=np.float32)
    Wh = np.asarray(Wh, dtype=np.float32)
    Wattn = np.asarray(Wattn, dtype=np.float32)
    b = np.asarray(b, dtype=np.float32)

    if "nc" not in _NC_CACHE:
        _NC_CACHE["nc"] = build_nc()
    nc = _NC_CACHE["nc"]

    in_maps = []
    for i in range(NCORES):
        sl = slice(i * NL, (i + 1) * NL)
        in_maps.append(_prep_core_inputs(x[sl], A[sl], Wx, Wh, Wattn, b))

    res = run_bass_kernel_spmd(
        nc, in_maps, core_ids=list(range(NCORES)), trace=_trace
    )
    outs = [res.results[i]["out"] for i in range(NCORES)]
    full = np.concatenate(outs, axis=0)  # (N, T, H)
    if _trace:
        kernel.last_exec_time_ns = res.exec_time_ns
        kernel.last_profile = res.profile_json
    return full


kernel.last_exec_time_ns = None
kernel.last_profile = None
